# revision 1
# baseline (speedup 1.0000x reference)
"""Trainium2 Bass kernel for nn_CMAModel (memory-augmented causal attention).

Sharding: 8 cores = 2 batches x 4 head-groups. Each core handles one batch and
4 heads (256 channels); the output projection is row-parallel and the 4
per-batch partials are summed on the host.

Per-core device program (all transposed, channels on partitions):
  qT = WqT.T @ xT, kT (incl. memory cols), V rows (S x per-head cols + ones col)
  gate logits from host-folded Wg = gate_w @ Wq; gsig = sigmoid
  per (head, T-chunk of 512): scoresT tiles [128S, 512T] -> exp (ACT, scale)
    -> causal tri-mask on diagonal blocks -> PV matmuls accumulate
    A_chunk/A_mem [65, 512] in PSUM; ones column gives row-sums (Z) for free
  combine: Y = (A_c + sig(gate)*A_m) / Z  via per-lane broadcasts (DMA)
  depthwise causal conv K=4 + residual + bias on [256ch, T]
  out partial [T, 1024] = R.T @ WoT  (PSUM -> DRAM)
"""
import contextlib
import ctypes
import os
import sys
import types

import numpy as np

# ---------------------------------------------------------------- constants
B, T, C = 2, 2048, 1024
H, HD = 16, 64
M = 256
G = 4                 # head-groups (cores per batch)
HPG = H // G          # 4 heads per core
CPG = HPG * HD        # 256 channels per core
S = T + 2 * M         # 2560 kv rows
SM = 2 * M            # 512 memory rows
NKT = C // 128        # 8 contraction tiles
NST = S // 128        # 20 S tiles (16 chunk + 4 mem)
TC = 512              # T chunk size
NTC = T // TC         # 4
SCALE = 1.0 / float(np.sqrt(HD))

_MM_DTYPE = os.environ.get("BASS_MM_DTYPE", "float32r")

_BUILT = None


# ------------------------------------------------------- axon NTFF hook shim
def _install_ntff_hook():
    """The agent image lacks antenv.axon_hooks; synthesize it so
    run_bass_kernel_spmd(trace=True) can capture NTFF profiles."""
    if "antenv.axon_hooks" in sys.modules:
        return
    so_path = "/opt/axon/libaxon_pjrt.so"
    hook = None
    if os.path.exists(so_path):
        try:
            lib = ctypes.CDLL(so_path)
            if hasattr(lib, "axon_start_nrt_profile"):
                lib.axon_start_nrt_profile.argtypes = [
                    ctypes.POINTER(ctypes.c_int64),
                    ctypes.c_size_t,
                ]
                lib.axon_start_nrt_profile.restype = ctypes.c_int64
                lib.axon_stop_nrt_profile.argtypes = [ctypes.c_char_p]
                lib.axon_stop_nrt_profile.restype = ctypes.c_int64

                @contextlib.contextmanager
                def _hook(output_dir, device_ids):
                    import jax

                    jax.devices()
                    if device_ids:
                        ids = (ctypes.c_int64 * len(device_ids))(*device_ids)
                        rc = lib.axon_start_nrt_profile(ids, len(device_ids))
                    else:
                        rc = lib.axon_start_nrt_profile(None, 0)
                    if rc != 0:
                        raise RuntimeError(f"axon_start_nrt_profile rc={rc}")
                    try:
                        yield
                    finally:
                        n = lib.axon_stop_nrt_profile(str(output_dir).encode())
                        if n < 0:
                            raise RuntimeError(f"axon_stop_nrt_profile rc={n}")

                hook = _hook
        except OSError:
            pass
    mod = types.ModuleType("antenv.axon_hooks")
    mod.get_axon_ntff_profile_hook = lambda: hook
    mod.set_axon_ntff_profile_hook = lambda h: None
    sys.modules["antenv.axon_hooks"] = mod


# ------------------------------------------------------------- device build
def _build_program():
    import concourse.tile as tile
    from concourse import bacc, mybir
    from concourse.masks import make_upper_triangular

    f32 = mybir.dt.float32
    mdt = getattr(mybir.dt, _MM_DTYPE)  # dtype of all matmul operands

    def mm(ap):
        return ap

    nc = bacc.Bacc("TRN2", target_bir_lowering=False, debug=False, num_devices=8)

    xT = nc.dram_tensor("xT", [C, T], mdt, kind="ExternalInput").ap()
    memT = nc.dram_tensor("memT", [C, SM], mdt, kind="ExternalInput").ap()
    WqT = nc.dram_tensor("WqT", [C, CPG], mdt, kind="ExternalInput").ap()
    WkT = nc.dram_tensor("WkT", [C, CPG], mdt, kind="ExternalInput").ap()
    WvTa = nc.dram_tensor("WvTa", [C, 65 * HPG], mdt, kind="ExternalInput").ap()
    WgT = nc.dram_tensor("WgT", [C, HPG], mdt, kind="ExternalInput").ap()
    gbn = nc.dram_tensor("gbn", [HPG, 1], f32, kind="ExternalInput").ap()
    WoT = nc.dram_tensor("WoT", [CPG, C], mdt, kind="ExternalInput").ap()
    K = 4
    cw = nc.dram_tensor("cw", [CPG, K], f32, kind="ExternalInput").ap()
    cb = nc.dram_tensor("cb", [CPG, 1], f32, kind="ExternalInput").ap()
    out = nc.dram_tensor("out", [T, C], f32, kind="ExternalOutput").ap()
    dbg_on = bool(int(os.environ.get("BASS_DBG", "0")))
    dbg = (
        nc.dram_tensor("dbg", [130, TC], f32, kind="ExternalOutput").ap()
        if dbg_on
        else None
    )

    Exp = mybir.ActivationFunctionType.Exp

    with tile.TileContext(nc) as tc:
        with contextlib.ExitStack() as ctx:
            const = ctx.enter_context(tc.tile_pool(name="const", bufs=1))
            xpool = ctx.enter_context(tc.tile_pool(name="xpool", bufs=2))
            sb = ctx.enter_context(tc.tile_pool(name="sb", bufs=1))
            work = ctx.enter_context(tc.tile_pool(name="work", bufs=3))
            small = ctx.enter_context(tc.tile_pool(name="small", bufs=1))
            psum = ctx.enter_context(
                tc.tile_pool(name="psum", bufs=1, space="PSUM")
            )
            drs = ctx.enter_context(tc.tile_pool(name="drs", bufs=4, space="DRAM"))

            # ---- constants / weights
            wq_s = const.tile([128, NKT, CPG], mdt)
            nc.sync.dma_start(out=wq_s, in_=WqT.rearrange("(a p) n -> p a n", p=128))
            wk_s = const.tile([128, NKT, CPG], mdt)
            nc.sync.dma_start(out=wk_s, in_=WkT.rearrange("(a p) n -> p a n", p=128))
            wva_s = const.tile([128, NKT, 65 * HPG], mdt)
            nc.sync.dma_start(out=wva_s, in_=WvTa.rearrange("(a p) n -> p a n", p=128))
            wg_s = const.tile([128, NKT, HPG], mdt)
            nc.sync.dma_start(out=wg_s, in_=WgT.rearrange("(a p) n -> p a n", p=128))
            wo_s = const.tile([128, 2, C], mdt)
            nc.sync.dma_start(out=wo_s, in_=WoT.rearrange("(a p) n -> p a n", p=128))
            cw_s = const.tile([128, 2, K], f32)
            nc.sync.dma_start(out=cw_s, in_=cw.rearrange("(a p) n -> p a n", p=128))
            cb_s = const.tile([128, 2, 1], f32)
            nc.sync.dma_start(out=cb_s, in_=cb.rearrange("(a p) n -> p a n", p=128))
            gbn_s = const.tile([HPG, 1], f32)
            nc.sync.dma_start(out=gbn_s, in_=gbn)

            tri = const.tile([128, 128], f32)
            make_upper_triangular(nc, tri, val=1.0, diag=True)

            # ---- persistent activations
            qT_s = sb.tile([128, 2, T], mdt)
            kT_s = sb.tile([128, 2, S], mdt)
            V_s = sb.tile([128, NST, 65 * HPG], mdt)  # [128, 20, 260]
            gsig = sb.tile([HPG, T], f32)

            xTr = xT.rearrange("(a p) t -> p a t", p=128)

            def proj_chunk(xh, tglob, tloc):
                """q/k/V/gate projections for T columns [tglob, tglob+512)."""
                for m in range(2):
                    pq = psum.tile([128, TC], f32, tag="ps", bufs=2)
                    for k in range(NKT):
                        nc.tensor.matmul(
                            pq,
                            mm(wq_s[:, k, m * 128:(m + 1) * 128]),
                            mm(xh[:, k, tloc:tloc + TC]),
                            start=(k == 0),
                            stop=(k == NKT - 1),
                        )
                    nc.vector.tensor_copy(qT_s[:, m, tglob:tglob + TC], pq)
                    pk = psum.tile([128, TC], f32, tag="ps", bufs=2)
                    for k in range(NKT):
                        nc.tensor.matmul(
                            pk,
                            mm(wk_s[:, k, m * 128:(m + 1) * 128]),
                            mm(xh[:, k, tloc:tloc + TC]),
                            start=(k == 0),
                            stop=(k == NKT - 1),
                        )
                    nc.vector.tensor_copy(kT_s[:, m, tglob:tglob + TC], pk)
                for mt in range(TC // 128):
                    st = tglob // 128 + mt
                    pv = psum.tile([128, 65 * HPG], f32, tag="ps", bufs=2)
                    for k in range(NKT):
                        nc.tensor.matmul(
                            pv,
                            mm(xh[:, k, tloc + mt * 128:tloc + (mt + 1) * 128]),
                            mm(wva_s[:, k, :]),
                            start=(k == 0),
                            stop=(k == NKT - 1),
                        )
                    nc.vector.tensor_copy(V_s[:, st, :], pv)
                    oc = V_s[:, st, 64:65 * HPG:65]
                    nc.vector.tensor_scalar(
                        oc, oc, 0.0, 1.0,
                        mybir.AluOpType.mult, mybir.AluOpType.add,
                    )
                pg = psum.tile([HPG, TC], f32, tag="ps", bufs=2)
                for k in range(NKT):
                    nc.tensor.matmul(
                        pg,
                        mm(wg_s[:, k, :]),
                        mm(xh[:, k, tloc:tloc + TC]),
                        start=(k == 0),
                        stop=(k == NKT - 1),
                    )
                # gsig <- exp(-(l + gate_b)) for now; finished below
                nc.scalar.activation(
                    gsig[:, tglob:tglob + TC], pg, Exp, bias=gbn_s, scale=-1.0
                )

            # T half 0
            xh0 = xpool.tile([128, NKT, T // 2], mdt, tag="xbig")
            for k in range(NKT):
                nc.sync.dma_start(out=xh0[:, k, :], in_=xTr[:, k, : T // 2])
            for ncn in range(2):
                proj_chunk(xh0, ncn * TC, ncn * TC)

            # memory projections
            mems = xpool.tile([128, NKT, SM], mdt, tag="xbig")
            nc.sync.dma_start(out=mems, in_=memT.rearrange("(a p) t -> p a t", p=128))
            for m in range(2):
                pk = psum.tile([128, SM], f32, tag="ps", bufs=2)
                for half in range(2):
                    for k in range(NKT):
                        nc.tensor.matmul(
                            pk[:, half * 256:(half + 1) * 256],
                            mm(wk_s[:, k, m * 128:(m + 1) * 128]),
                            mm(mems[:, k, half * 256:(half + 1) * 256]),
                            start=(k == 0),
                            stop=(k == NKT - 1),
                        )
                nc.vector.tensor_copy(kT_s[:, m, T:], pk)
            for mt in range(SM // 128):
                st = 16 + mt
                pv = psum.tile([128, 65 * HPG], f32, tag="ps", bufs=2)
                for k in range(NKT):
                    nc.tensor.matmul(
                        pv,
                        mm(mems[:, k, mt * 128:(mt + 1) * 128]),
                        mm(wva_s[:, k, :]),
                        start=(k == 0),
                        stop=(k == NKT - 1),
                    )
                nc.vector.tensor_copy(V_s[:, st, :], pv)
                oc = V_s[:, st, 64:65 * HPG:65]
                nc.vector.tensor_scalar(
                    oc, oc, 0.0, 1.0,
                    mybir.AluOpType.mult, mybir.AluOpType.add,
                )

            # T half 1
            xh1 = xpool.tile([128, NKT, T // 2], mdt, tag="xbig")
            for k in range(NKT):
                nc.sync.dma_start(out=xh1[:, k, :], in_=xTr[:, k, T // 2:])
            for ncn in range(2):
                proj_chunk(xh1, T // 2 + ncn * TC, ncn * TC)

            # finish sigmoid: gsig = 1 / (1 + exp(-(l+b)))
            nc.vector.tensor_scalar_add(gsig, gsig, 1.0)
            nc.vector.reciprocal(gsig, gsig)

            # ---- attention + combine; attnout[:, 0:2] = Y pairs, [:, 2:4] = conv out
            attnout = xpool.tile([128, 4, T], mdt, tag="xbig")

            def conv_pair(p):
                """depthwise causal conv + residual + bias on GpSimd
                (idle engine; keeps DVE stream free for combines)."""
                ctmp = small.tile([128, T], f32, tag="ctmp", bufs=2, name=f"ctmp{p}")
                y = attnout[:, p, :]
                R = attnout[:, 2 + p, :]
                nc.vector.tensor_scalar_add(R, y, cb_s[:, p, :])
                for k in range(K):
                    sh = K - 1 - k
                    if sh == 0:
                        nc.vector.tensor_scalar_mul(ctmp, y, cw_s[:, p, k:k + 1])
                        nc.vector.tensor_add(R, R, ctmp)
                    else:
                        nc.vector.tensor_scalar_mul(
                            ctmp[:, sh:], y[:, :T - sh], cw_s[:, p, k:k + 1]
                        )
                        nc.vector.tensor_add(R[:, sh:], R[:, sh:], ctmp[:, sh:])

            for hl in range(HPG):
                mq, par = divmod(hl, 2)
                ro = 64 * par
                vc = 65 * hl
                for j in range(NTC):
                    nct = 4 * (j + 1)   # visible chunk S-tiles
                    Ac = psum.tile([128, TC], f32, tag="pa", bufs=6)
                    Am = psum.tile([128, TC], f32, tag="pa", bufs=6)
                    for i in range(nct + 4):
                        is_mem = i >= nct
                        si = (16 + i - nct) if is_mem else i
                        off = 0
                        if not is_mem and si >= 4 * j:
                            off = 128 * si - TC * j
                        n = TC - off
                        ps = psum.tile([128, TC], f32, tag="ps", bufs=2)
                        nc.tensor.matmul(
                            ps[:, off:],
                            mm(kT_s[ro:ro + 64, mq, si * 128:(si + 1) * 128]),
                            mm(qT_s[ro:ro + 64, mq, TC * j + off:TC * (j + 1)]),
                            start=True,
                            stop=True,
                        )
                        Pt = work.tile([128, TC], mdt, tag="P")
                        nc.scalar.activation(
                            Pt[:, off:], ps[:, off:], Exp, scale=SCALE
                        )
                        if not is_mem and si >= 4 * j:
                            nc.vector.tensor_mul(
                                Pt[:, off:off + 128], Pt[:, off:off + 128], tri
                            )
                        dst = Am if is_mem else Ac
                        first = (i == 0) or (is_mem and i == nct)
                        last = (i == nct - 1) or (i == nct + 3)
                        nc.tensor.matmul(
                            dst[0:65, off:],
                            mm(V_s[:, si, vc:vc + 65]),
                            mm(Pt[:, off:]),
                            start=first,
                            stop=last,
                        )
                    if dbg is not None and hl == 0 and j == 0:
                        dbt = small.tile([65, 2, TC], f32, tag="dbt", bufs=1)
                        nc.vector.tensor_copy(dbt[:, 0, :], Ac[0:65, :])
                        nc.vector.tensor_copy(dbt[:, 1, :], Am[0:65, :])
                        nc.sync.dma_start(out=dbg[0:65, :], in_=dbt[:, 0, :])
                        nc.sync.dma_start(out=dbg[65:130, :], in_=dbt[:, 1, :])
                    # combine: Y = (Ac + g*Am) / Z on lanes 0..63, Z at lane 64
                    zu = small.tile([128, TC], f32, tag="zu", bufs=2)
                    nc.vector.tensor_copy(zu[64:65, :], Ac[64:65, :])
                    nc.vector.tensor_add(
                        zu[64:65, :], zu[64:65, :], Am[64:65, :]
                    )
                    # reshape Z row and gate row onto 128 lanes so reciprocal
                    # runs 128-wide (4 elems/lane) instead of 512 on one lane
                    zrg = small.tile([128, 2, TC // 128], f32, tag="zrg", bufs=2)
                    nc.sync.dma_start(out=zrg[:, 0, :], in_=zu[64:65, :])
                    nc.sync.dma_start(
                        out=zrg[:, 1, :], in_=gsig[hl:hl + 1, TC * j:TC * (j + 1)]
                    )
                    nc.vector.reciprocal(zrg[:, 0, :], zrg[:, 0, :])
                    nc.vector.tensor_mul(zrg[:, 1, :], zrg[:, 1, :], zrg[:, 0, :])
                    # bounce through DRAM so the rows can be partition-broadcast
                    zsc = drs.tile([2, TC], f32, tag="zsc", bufs=4)
                    nc.sync.dma_start(out=zsc[0:1, :], in_=zrg[:, 0, :])
                    nc.sync.dma_start(out=zsc[1:2, :], in_=zrg[:, 1, :])
                    rzb = small.tile([64, TC], f32, tag="bc", bufs=4)
                    nc.sync.dma_start(
                        out=rzb, in_=zsc[0:1, :].partition_broadcast(64)
                    )
                    gzb = small.tile([64, TC], f32, tag="bc", bufs=4)
                    nc.sync.dma_start(
                        out=gzb, in_=zsc[1:2, :].partition_broadcast(64)
                    )
                    tmp = small.tile([64, TC], f32, tag="tmp", bufs=2)
                    tmp2 = small.tile([64, TC], mdt, tag="tmp", bufs=2)
                    nc.vector.tensor_mul(tmp, Am[0:64, :], gzb)   # Am * g/Z
                    nc.vector.tensor_mul(tmp2, Ac[0:64, :], rzb)  # Ac / Z
                    if par == 0:
                        nc.vector.tensor_add(
                            attnout[0:64, mq, TC * j:TC * (j + 1)], tmp, tmp2
                        )
                    else:
                        nc.vector.tensor_add(tmp2, tmp, tmp2)
                        nc.sync.dma_start(
                            out=attnout[64:128, mq, TC * j:TC * (j + 1)], in_=tmp2
                        )
                if hl == 1 or hl == 3:
                    conv_pair(hl // 2)

            # ---- output projection: out[T, C] partial
            for mt in range(T // 128):
                for nb in range(2):
                    po = psum.tile([128, TC], f32, tag="ps", bufs=2)
                    for p in range(2):
                        nc.tensor.matmul(
                            po,
                            mm(attnout[:, 2 + p, mt * 128:(mt + 1) * 128]),
                            mm(wo_s[:, p, nb * TC:(nb + 1) * TC]),
                            start=(p == 0),
                            stop=(p == 1),
                        )
                    ot = work.tile([128, TC], f32, tag="ot", bufs=3)
                    nc.any.tensor_copy(ot, po)
                    nc.sync.dma_start(
                        out=out[mt * 128:(mt + 1) * 128, nb * TC:(nb + 1) * TC],
                        in_=ot,
                    )

    nc.compile()
    return nc


def _get_program():
    global _BUILT
    if _BUILT is None:
        _install_ntff_hook()
        _BUILT = _build_program()
    return _BUILT


# --------------------------------------------------------------- host side
def _tf32_round(a):
    """Cast to the matmul-operand dtype: TF32-round for float32r (data stays
    fp32 bits), bfloat16 for bf16 mode, passthrough for float32."""
    if _MM_DTYPE == "bfloat16":
        import ml_dtypes

        return np.ascontiguousarray(a, np.float32).astype(ml_dtypes.bfloat16)
    if _MM_DTYPE != "float32r":
        return np.ascontiguousarray(a, np.float32)
    u = np.ascontiguousarray(a, np.float32).view(np.uint32).astype(np.uint64)
    u = (u + 0x0FFF + ((u >> 13) & 1)) & np.uint64(0xFFFFE000)
    return u.astype(np.uint32).view(np.float32)


def host_prep(inputs):
    x = np.ascontiguousarray(np.asarray(inputs["x"], np.float32))
    fwd = np.asarray(inputs["fwd_mem"], np.float32)
    rev = np.asarray(inputs["rev_mem"], np.float32)
    Wq = np.asarray(inputs["Wq"], np.float32)
    Wk = np.asarray(inputs["Wk"], np.float32)
    Wv = np.asarray(inputs["Wv"], np.float32)
    Wo = np.asarray(inputs["Wo"], np.float32)
    gate_w = np.asarray(inputs["gate_w"], np.float32)
    gate_b = np.asarray(inputs["gate_b"], np.float32)
    canon_w = np.asarray(inputs["canon_w"], np.float32)
    canon_bias = np.asarray(inputs["canon_bias"], np.float32)

    Wg = (gate_w.astype(np.float64) @ Wq.astype(np.float64)).astype(np.float32)

    per_b, per_g = [], []
    for b in range(B):
        per_b.append({
            "xT": _tf32_round(x[b].T),
            "memT": _tf32_round(np.concatenate([fwd[b], rev[b]], axis=0).T),
        })
    for g in range(G):
        cs = slice(g * CPG, (g + 1) * CPG)
        WvTa = np.zeros((C, 65 * HPG), np.float32)
        for h in range(HPG):
            rows = Wv[g * CPG + h * HD: g * CPG + (h + 1) * HD]
            WvTa[:, 65 * h:65 * h + 64] = rows.T
        hs = slice(g * HPG, (g + 1) * HPG)
        per_g.append({
            "WqT": _tf32_round(Wq[cs].T),
            "WkT": _tf32_round(Wk[cs].T),
            "WvTa": _tf32_round(WvTa),
            "WgT": _tf32_round(Wg[hs].T),
            "gbn": np.ascontiguousarray(-gate_b[hs]).reshape(HPG, 1),
            "WoT": _tf32_round(Wo[:, cs].T),
            "cw": np.ascontiguousarray(canon_w[cs, 0, :]),
            "cb": np.ascontiguousarray(canon_bias[cs]).reshape(CPG, 1),
        })
    return per_b, per_g


LAST_EXEC_NS = None
LAST_RESULTS = None


def kernel(**inputs):
    global LAST_EXEC_NS, LAST_RESULTS
    from concourse.bass_utils import run_bass_kernel_spmd

    nc = _get_program()
    per_b, per_g = host_prep(inputs)
    in_maps = []
    for core in range(8):
        b, g = divmod(core, G)
        m = {}
        m.update(per_b[b])
        m.update(per_g[g])
        in_maps.append(m)

    trace = bool(int(os.environ.get("KERNEL_TRACE", "0")))
    kw = {}
    if trace:
        tcores = os.environ.get("KERNEL_TRACE_CORES", "0")
        kw = dict(
            trace=True,
            trace_cores=[int(c) for c in tcores.split(",")],
            tmpdir=os.environ.get("KERNEL_TRACE_DIR", None),
        )
    res = run_bass_kernel_spmd(nc, in_maps, core_ids=list(range(8)), **kw)
    LAST_EXEC_NS = res.exec_time_ns
    LAST_RESULTS = res
    outp = np.zeros((B, T, C), np.float32)
    for core in range(8):
        b = core // G
        outp[b] += res.results[core]["out"]
    return outp



# revision 3
# speedup vs baseline: 1.8376x; 1.8376x over previous
"""Trainium2 Bass kernel for nn_CMAModel (memory-augmented causal attention).

v2: chunk-pipelined schedule tuned for the PE HAM clock gate.

Sharding: 8 cores = 2 batches x 4 head-groups; per core 4 heads as two
pairs (mq=0: heads 0,1 on partitions 0-63/64-127; mq=1: heads 2,3),
out_proj row-parallel, per-batch partials summed on host.

All matmul operands bf16 (FWL weight loads, half DMA). Schedule:
  phase A: proj chunk 0, memory k/V, gate logits+sigmoid for all chunks
  per chunk j: attention slots for both pairs, with proj of chunk j+1 and
    out-proj of chunk j-1 interleaved as PE fillers so the Tensor engine
    never idles (keeps the HAM clock gate at 8/8 = 2.4 GHz); one scalar
    Exp per slot covers both heads' score tiles (two PSUM banks, one
    instruction); combine evacuates accumulators to SBUF promptly to
    release PSUM, then normalizes via DMA partition-broadcast of 1/Z and
    g/Z; depthwise conv (residual folded into tap 3) on vector engine.
"""
import contextlib
import ctypes
import os
import sys
import types

import numpy as np

# ---------------------------------------------------------------- constants
B, T, C = 2, 2048, 1024
H, HD = 16, 64
M = 256
G = 4                 # head-groups (cores per batch)
HPG = H // G          # 4 heads per core
CPG = HPG * HD        # 256 channels per core
S = T + 2 * M         # 2560 kv rows
SM = 2 * M            # 512 memory rows
NKT = C // 128        # 8 contraction tiles
TC = 512              # T chunk size
NTC = T // TC         # 4
KCONV = 4
SCALE = 1.0 / float(np.sqrt(HD))

# broadcast path for 1/Z rows: "sbuf" = SBUF->SBUF DMA partition_broadcast,
# "dram" = bounce through DRAM (baseline-proven)
_BCAST = os.environ.get("BASS_BCAST", "dram")
_FILLERS_PER_SLOT = int(os.environ.get("BASS_FPS", "1"))

_BUILT = None


# ------------------------------------------------------- axon NTFF hook shim
def _install_ntff_hook():
    if "antenv.axon_hooks" in sys.modules:
        return
    so_path = "/opt/axon/libaxon_pjrt.so"
    hook = None
    if os.path.exists(so_path):
        try:
            lib = ctypes.CDLL(so_path)
            if hasattr(lib, "axon_start_nrt_profile"):
                lib.axon_start_nrt_profile.argtypes = [
                    ctypes.POINTER(ctypes.c_int64),
                    ctypes.c_size_t,
                ]
                lib.axon_start_nrt_profile.restype = ctypes.c_int64
                lib.axon_stop_nrt_profile.argtypes = [ctypes.c_char_p]
                lib.axon_stop_nrt_profile.restype = ctypes.c_int64

                @contextlib.contextmanager
                def _hook(output_dir, device_ids):
                    import jax

                    jax.devices()
                    if device_ids:
                        ids = (ctypes.c_int64 * len(device_ids))(*device_ids)
                        rc = lib.axon_start_nrt_profile(ids, len(device_ids))
                    else:
                        rc = lib.axon_start_nrt_profile(None, 0)
                    if rc != 0:
                        raise RuntimeError(f"axon_start_nrt_profile rc={rc}")
                    try:
                        yield
                    finally:
                        n = lib.axon_stop_nrt_profile(str(output_dir).encode())
                        if n < 0:
                            raise RuntimeError(f"axon_stop_nrt_profile rc={n}")

                hook = _hook
        except OSError:
            pass
    mod = types.ModuleType("antenv.axon_hooks")
    mod.get_axon_ntff_profile_hook = lambda: hook
    mod.set_axon_ntff_profile_hook = lambda h: None
    sys.modules["antenv.axon_hooks"] = mod


# ------------------------------------------------------------- device build
def _build_program():
    import concourse.tile as tile
    from concourse import bacc, mybir
    from concourse.masks import make_upper_triangular

    f32 = mybir.dt.float32
    bf16 = mybir.dt.bfloat16
    Exp = mybir.ActivationFunctionType.Exp
    mult = mybir.AluOpType.mult
    add = mybir.AluOpType.add

    nc = bacc.Bacc("TRN2", target_bir_lowering=False, debug=False, num_devices=8)

    xT = nc.dram_tensor("xT", [C, T], bf16, kind="ExternalInput").ap()
    memT = nc.dram_tensor("memT", [C, SM], bf16, kind="ExternalInput").ap()
    WqT = nc.dram_tensor("WqT", [C, CPG], bf16, kind="ExternalInput").ap()
    WkT = nc.dram_tensor("WkT", [C, CPG], bf16, kind="ExternalInput").ap()
    WvTa = nc.dram_tensor("WvTa", [C, 65 * HPG], bf16, kind="ExternalInput").ap()
    WgT = nc.dram_tensor("WgT", [C, HPG], bf16, kind="ExternalInput").ap()
    gbn = nc.dram_tensor("gbn", [HPG, 1], f32, kind="ExternalInput").ap()
    WoT = nc.dram_tensor("WoT", [CPG, C], bf16, kind="ExternalInput").ap()
    cw = nc.dram_tensor("cw", [CPG, KCONV], f32, kind="ExternalInput").ap()
    cb = nc.dram_tensor("cb", [CPG, 1], f32, kind="ExternalInput").ap()
    out = nc.dram_tensor("out", [T, C], bf16, kind="ExternalOutput").ap()
    outr = out.rearrange("t (b n) -> t b n", b=2)

    with tile.TileContext(nc) as tc:
        with contextlib.ExitStack() as ctx:
            const = ctx.enter_context(tc.tile_pool(name="const", bufs=1))
            sb = ctx.enter_context(tc.tile_pool(name="sb", bufs=1))
            work = ctx.enter_context(tc.tile_pool(name="work", bufs=1))
            psum = ctx.enter_context(
                tc.tile_pool(name="psum", bufs=1, space="PSUM")
            )
            drs = ctx.enter_context(
                tc.tile_pool(name="drs", bufs=4, space="DRAM")
            )

            # ---- weights + inputs (DMA order = first-use order)
            wq_s = const.tile([128, NKT, CPG], bf16)
            nc.sync.dma_start(out=wq_s, in_=WqT.rearrange("(a p) n -> p a n", p=128))
            wk_s = const.tile([128, NKT, CPG], bf16)
            nc.sync.dma_start(out=wk_s, in_=WkT.rearrange("(a p) n -> p a n", p=128))
            xs = sb.tile([128, NKT, T], bf16)
            xTr = xT.rearrange("(a p) t -> p a t", p=128)
            nc.sync.dma_start(out=xs[:, 0:2, 0:TC], in_=xTr[:, 0:2, 0:TC])
            nc.sync.dma_start(out=xs[:, 2:NKT, 0:TC], in_=xTr[:, 2:NKT, 0:TC])
            wva_s = const.tile([128, NKT, 65 * HPG], bf16)
            nc.sync.dma_start(out=wva_s, in_=WvTa.rearrange("(a p) n -> p a n", p=128))
            mems = sb.tile([128, NKT, SM], bf16)
            nc.sync.dma_start(out=mems, in_=memT.rearrange("(a p) t -> p a t", p=128))
            wg_s = const.tile([128, NKT, HPG], bf16)
            nc.sync.dma_start(out=wg_s, in_=WgT.rearrange("(a p) n -> p a n", p=128))
            for c in range(1, NTC):
                cs = slice(c * TC, (c + 1) * TC)
                nc.sync.dma_start(out=xs[:, :, cs], in_=xTr[:, :, cs])
            wo_s = const.tile([128, 2, C], bf16)
            nc.sync.dma_start(out=wo_s, in_=WoT.rearrange("(a p) n -> p a n", p=128))
            cw_s = const.tile([128, 2, KCONV], f32)
            nc.sync.dma_start(out=cw_s, in_=cw.rearrange("(a p) n -> p a n", p=128))
            cb_s = const.tile([128, 2, 1], f32)
            nc.sync.dma_start(out=cb_s, in_=cb.rearrange("(a p) n -> p a n", p=128))
            gbn_s = const.tile([HPG, 1], f32)
            nc.sync.dma_start(out=gbn_s, in_=gbn)

            # causal mask for diagonal blocks, duplicated for both heads
            tri = const.tile([128, 128], f32)
            make_upper_triangular(nc, tri, val=1.0, diag=True)
            tri2 = const.tile([128, 2, 128], f32)
            nc.vector.tensor_copy(tri2[:, 0, :], tri)
            nc.vector.tensor_copy(tri2[:, 1, :], tri)

            qT_s = sb.tile([128, 2, T], bf16)
            kT_s = sb.tile([128, 2, S], bf16)
            V_s = sb.tile([128, S // 128, 65 * HPG], bf16)
            # e = exp(-(logit+bias)) per head; sigmoid is folded into the
            # combine's reciprocal: g/Z = 1/(Z + Z*e)
            gE = sb.tile([HPG, T], f32)
            # e rows reshaped to 128 partitions: [128, head-in-pair, pair,
            # NTC*4] so combine ops run full-width
            ew = sb.tile([128, 2, 2, NTC * 4], f32)
            Y_s = sb.tile([128, 2, T], bf16)
            R_s = sb.tile([128, 2, T], bf16)

            # ---------------- building blocks -------------------------
            def proj_qk(ws, dst, mq, tglob, src, sloc, n):
                """dst[:, mq, tglob:tglob+n] = ws[:,:,mq-half].T @ src cols."""
                ps = psum.tile([128, 2, TC], f32, tag="ps", bufs=2, name="psq")
                for k in range(NKT):
                    nc.tensor.matmul(
                        ps[:, 0, :n],
                        ws[:, k, mq * 128:(mq + 1) * 128],
                        src[:, k, sloc:sloc + n],
                        start=(k == 0),
                        stop=(k == NKT - 1),
                    )
                nc.vector.tensor_copy(dst[:, mq, tglob:tglob + n], ps[:, 0, :n])

            def proj_v(st, src, sloc):
                """V_s[:, st, :] = src[:, :, sloc:+128].T @ WvTa (+ ones cols)."""
                ps = psum.tile([128, 2, TC], f32, tag="ps", bufs=2, name="psv")
                pv = ps[:, 0, : 65 * HPG]
                for k in range(NKT):
                    nc.tensor.matmul(
                        pv,
                        src[:, k, sloc:sloc + 128],
                        wva_s[:, k, :],
                        start=(k == 0),
                        stop=(k == NKT - 1),
                    )
                nc.vector.tensor_copy(V_s[:, st, :], pv)
                oc = V_s[:, st, 64:65 * HPG:65]
                nc.vector.tensor_scalar(oc, oc, 0.0, 1.0, mult, add)

            def proj_gate(cn):
                """gsig[:, chunk cn] = sigmoid(gate logits) via exp + 1/(1+e)."""
                tglob = cn * TC
                ps = psum.tile([128, 2, TC], f32, tag="ps", bufs=2, name="psg")
                pg = ps[0:HPG, 0, :]
                for k in range(NKT):
                    nc.tensor.matmul(
                        pg,
                        wg_s[:, k, :],
                        xs[:, k, tglob:tglob + TC],
                        start=(k == 0),
                        stop=(k == NKT - 1),
                    )
                gsl = gE[:, tglob:tglob + TC]
                nc.scalar.activation(gsl, pg, Exp, bias=gbn_s, scale=-1.0)
                # reshape e rows to 128-partition layout via DRAM bounce
                gdr = drs.tile([HPG, TC], f32, tag="gdr", bufs=2, name="gdr")
                nc.sync.dma_start(out=gdr, in_=gsl)
                for mq in range(2):
                    nc.sync.dma_start(
                        out=ew[:, :, mq, cn * 4:cn * 4 + 4],
                        in_=gdr[2 * mq:2 * mq + 2, :].rearrange(
                            "a (p c) -> p a c", p=128
                        ),
                    )

            def outproj_mt(mt):
                """out rows [mt*128, +128) = R.T @ WoT, both 512-col halves."""
                po = psum.tile([128, 2, TC], f32, tag="ps", bufs=2, name="po")
                for nb in range(2):
                    for p in range(2):
                        nc.tensor.matmul(
                            po[:, nb, :],
                            R_s[:, p, mt * 128:(mt + 1) * 128],
                            wo_s[:, p, nb * TC:(nb + 1) * TC],
                            start=(p == 0),
                            stop=(p == 1),
                        )
                ot = work.tile([128, 2, TC], bf16, tag="ot", bufs=3, name="ot")
                nc.vector.tensor_copy(ot[:, 0, :], po[:, 0, :])
                nc.scalar.copy(ot[:, 1, :], po[:, 1, :])
                nc.sync.dma_start(
                    out=outr[mt * 128:(mt + 1) * 128, :, :], in_=ot
                )

            def conv_chunk(j):
                """causal depthwise conv K=4, residual folded into tap 3."""
                a, b = j * TC, (j + 1) * TC
                for m in range(2):
                    # R = y*cw3' + cb  (cw3' = cw[3]+1 folds the residual)
                    nc.vector.tensor_scalar(
                        R_s[:, m, a:b], Y_s[:, m, a:b],
                        cw_s[:, m, 3:4], cb_s[:, m, :], mult, add,
                    )
                    for k in range(KCONV - 1):
                        sh = KCONV - 1 - k
                        lo = max(a - sh, 0)
                        nc.vector.scalar_tensor_tensor(
                            R_s[:, m, lo + sh:b],
                            Y_s[:, m, lo:b - sh],
                            cw_s[:, m, k:k + 1],
                            R_s[:, m, lo + sh:b],
                            mult, add,
                        )

            # fillers: closures giving the scheduler dense PE work to slot
            # between attention matmuls
            filler_q = []

            def pop_fillers(nmax):
                for _ in range(nmax):
                    if not filler_q:
                        return
                    filler_q.pop(0)()

            def fill_q(cn):
                return [
                    (lambda mq=mq: proj_qk(wq_s, qT_s, mq, cn * TC, xs,
                                           cn * TC, TC))
                    for mq in range(2)
                ]

            def fill_k(cn):
                return [
                    (lambda mq=mq: proj_qk(wk_s, kT_s, mq, cn * TC, xs,
                                           cn * TC, TC))
                    for mq in range(2)
                ]

            def fill_v(cn):
                return [
                    (lambda st=cn * 4 + mt, sl=cn * TC + mt * 128:
                     proj_v(st, xs, sl))
                    for mt in range(TC // 128)
                ]

            def fill_o(cn):
                return [
                    (lambda mt=mt: outproj_mt(mt))
                    for mt in range(cn * 4, cn * 4 + 4)
                ]

            # ---------------- phase A ---------------------------------
            for mq in range(2):
                proj_qk(wq_s, qT_s, mq, 0, xs, 0, TC)
                proj_qk(wk_s, kT_s, mq, 0, xs, 0, TC)
            for mt in range(TC // 128):
                proj_v(mt, xs, mt * 128)
            for mq in range(2):          # memory keys -> kT_s[:, mq, T:]
                ps = psum.tile([128, 2, TC], f32, tag="ps", bufs=2, name="psm")
                for k in range(NKT):
                    nc.tensor.matmul(
                        ps[:, 0, :],
                        wk_s[:, k, mq * 128:(mq + 1) * 128],
                        mems[:, k, :],
                        start=(k == 0),
                        stop=(k == NKT - 1),
                    )
                nc.vector.tensor_copy(kT_s[:, mq, T:], ps[:, 0, :])
            for mt in range(SM // 128):  # memory values
                proj_v(T // 128 + mt, mems, mt * 128)
            proj_gate(0)

            # per-block filler lists: (early, paced). Early fillers pop one
            # per slot from slot 0 (chunk 3's k/V are read mid-block-3, so
            # they must land within the first ~10 slots). Paced fillers are
            # spread evenly across the block — front-loading makes late
            # slots PE-starved and pops conv-dependent out-proj work before
            # the vector queue has drained.
            block_fillers = [
                ([], fill_q(1) + fill_k(1) + fill_v(1)
                 + [lambda: proj_gate(1)]),
                ([], fill_q(2) + fill_k(2) + fill_v(2)
                 + [lambda: proj_gate(2)] + fill_o(0)),
                ([], fill_q(3) + [lambda: proj_gate(3)] + fill_o(1)),
                (fill_k(3) + fill_v(3), fill_o(2)),
            ]

            # ---------------- attention chunks ------------------------
            for j in range(NTC):
                early_q, paced = block_fillers[j]
                filler_q.extend(paced)
                nct = 4 * (j + 1)
                slots_total = 2 * (nct + 4)
                nfill = len(filler_q)
                slot_ctr = popped = 0
                for mq in range(2):
                    acc = [
                        psum.tile([128, TC], f32, tag="pa", bufs=4,
                                  name=f"acc{mq}{j}{x}")
                        for x in range(4)           # AcA, AmA, AcB, AmB
                    ]
                    for i in range(nct + 4):
                        is_mem = i >= nct
                        si = (T // 128 + i - nct) if is_mem else i
                        off = 0
                        if not is_mem and si >= 4 * j:
                            off = 128 * si - TC * j
                        ps = psum.tile([128, 2, TC], f32, tag="ps", bufs=2,
                                       name="pss")
                        for a in range(2):
                            nc.tensor.matmul(
                                ps[:, a, off:],
                                kT_s[64 * a:64 * a + 64, mq,
                                     si * 128:(si + 1) * 128],
                                qT_s[64 * a:64 * a + 64, mq,
                                     TC * j + off:TC * (j + 1)],
                                start=True,
                                stop=True,
                            )
                        Pt = work.tile([128, 2, TC], bf16, tag="P", bufs=3,
                                       name="Pt")
                        nc.scalar.activation(
                            Pt[:, :, off:], ps[:, :, off:], Exp, scale=SCALE
                        )
                        if not is_mem and si >= 4 * j:
                            doff = 128 * si - TC * j
                            nc.vector.tensor_mul(
                                Pt[:, :, doff:doff + 128],
                                Pt[:, :, doff:doff + 128],
                                tri2,
                            )
                        # PE filler between QK and PV: the PV waits on the
                        # scalar exp, so give the PE independent work here
                        slot_ctr += 1
                        if early_q:
                            early_q.pop(0)()
                        while popped < (slot_ctr * nfill) // slots_total:
                            pop_fillers(1)
                            popped += 1
                        for a in range(2):
                            dst = acc[2 * a + (1 if is_mem else 0)]
                            first = (i == 0) or (is_mem and i == nct)
                            last = (i == nct - 1) or (i == nct + 3)
                            nc.tensor.matmul(
                                dst[0:65, off:],
                                V_s[:, si, 65 * (2 * mq + a):
                                    65 * (2 * mq + a) + 65],
                                Pt[:, a, off:],
                                start=first,
                                stop=last,
                            )

                    # ---- combine: evacuate PSUM, normalize, gate ----
                    # (DVE ops need all operands on the same partitions, so
                    # Z math stays on partition 64, DMA aligns rows, and
                    # head B's result is DMA-moved into partitions 64-127)
                    acs = work.tile([128, 4, TC], f32, tag="acs", bufs=2,
                                    name="acs")
                    for x in range(4):
                        nc.vector.tensor_copy(acs[0:65, x, :], acc[x][0:65, :])
                    # Z rows -> DRAM -> [128, a, {Zc,Zm}, 4] full-width layout
                    zdraw = drs.tile([4, TC], f32, tag="zdraw", bufs=4,
                                     name="zdraw")
                    nc.sync.dma_start(out=zdraw, in_=acs[64:65, :, :])
                    zw = work.tile([128, 2, 2, 4], f32, tag="zw", bufs=2,
                                   name="zw")
                    for a in range(2):
                        nc.sync.dma_start(
                            out=zw[:, a, :, :],
                            in_=zdraw[2 * a:2 * a + 2, :].rearrange(
                                "q (p c) -> p q c", p=128
                            ),
                        )
                    # zf plane 0 = 1/Z, plane 1 = g/Z = 1/(Z + Z*e)
                    zf = work.tile([128, 2, 2, 4], f32, tag="zf", bufs=2,
                                   name="zf")
                    nc.vector.tensor_add(
                        zf[:, :, 0, :], zw[:, :, 0, :], zw[:, :, 1, :]
                    )
                    nc.vector.tensor_mul(
                        zf[:, :, 1, :], zf[:, :, 0, :],
                        ew[:, :, mq, 4 * j:4 * (j + 1)],
                    )
                    nc.vector.tensor_add(
                        zf[:, :, 1, :], zf[:, :, 1, :], zf[:, :, 0, :]
                    )
                    nc.vector.reciprocal(zf, zf)
                    zdr = drs.tile([1, 2, 2, TC], f32, tag="zdr", bufs=4,
                                   name="zdr")
                    for a in range(2):
                        nc.sync.dma_start(
                            out=zdr[0:1, a, :, :].rearrange(
                                "o q (p c) -> (o p) q c", p=128
                            ),
                            in_=zf[:, a, :, :],
                        )
                    # bcast planes: [64, head, rz/gz, TC], all at parts 0-63
                    bcast = work.tile([64, 2, 2, TC], f32, tag="bc", bufs=2,
                                      name="bcast")
                    nc.sync.dma_start(
                        out=bcast, in_=zdr.partition_broadcast(64)
                    )
                    ytmp = work.tile([64, 3, TC], f32, tag="yt", bufs=2,
                                     name="ytmp")
                    for a in range(2):
                        nc.vector.tensor_mul(
                            ytmp[:, 0, :], acs[0:64, 2 * a, :],
                            bcast[:, a, 0, :],
                        )
                        nc.vector.tensor_mul(
                            ytmp[:, 1, :], acs[0:64, 2 * a + 1, :],
                            bcast[:, a, 1, :],
                        )
                        if a == 0:
                            nc.vector.tensor_add(
                                Y_s[0:64, mq, TC * j:TC * (j + 1)],
                                ytmp[:, 0, :], ytmp[:, 1, :],
                            )
                        else:
                            yb = work.tile([64, TC], bf16, tag="yb", bufs=2,
                                           name="yb")
                            nc.vector.tensor_add(yb, ytmp[:, 0, :],
                                                 ytmp[:, 1, :])
                            nc.sync.dma_start(
                                out=Y_s[64:128, mq, TC * j:TC * (j + 1)],
                                in_=yb,
                            )
                conv_chunk(j)

            # drain remaining fillers, then final chunk's output rows
            pop_fillers(len(filler_q))
            for mt in range((NTC - 1) * 4, NTC * 4):
                outproj_mt(mt)

    nc.compile()
    return nc


def _get_program():
    global _BUILT
    if _BUILT is None:
        _install_ntff_hook()
        _BUILT = _build_program()
    return _BUILT


# --------------------------------------------------------------- host side
def _bf16(a):
    import ml_dtypes

    return np.ascontiguousarray(np.asarray(a, np.float32)).astype(
        ml_dtypes.bfloat16
    )


def host_prep(inputs):
    x = np.asarray(inputs["x"], np.float32)
    fwd = np.asarray(inputs["fwd_mem"], np.float32)
    rev = np.asarray(inputs["rev_mem"], np.float32)
    Wq = np.asarray(inputs["Wq"], np.float32)
    Wk = np.asarray(inputs["Wk"], np.float32)
    Wv = np.asarray(inputs["Wv"], np.float32)
    Wo = np.asarray(inputs["Wo"], np.float32)
    gate_w = np.asarray(inputs["gate_w"], np.float32)
    gate_b = np.asarray(inputs["gate_b"], np.float32)
    canon_w = np.asarray(inputs["canon_w"], np.float32)
    canon_bias = np.asarray(inputs["canon_bias"], np.float32)

    Wg = (gate_w.astype(np.float64) @ Wq.astype(np.float64)).astype(np.float32)

    per_b, per_g = [], []
    for b in range(B):
        per_b.append({
            "xT": _bf16(x[b].T),
            "memT": _bf16(np.concatenate([fwd[b], rev[b]], axis=0).T),
        })
    for g in range(G):
        cs = slice(g * CPG, (g + 1) * CPG)
        WvTa = np.zeros((C, 65 * HPG), np.float32)
        for h in range(HPG):
            rows = Wv[g * CPG + h * HD: g * CPG + (h + 1) * HD]
            WvTa[:, 65 * h:65 * h + 64] = rows.T
        hs = slice(g * HPG, (g + 1) * HPG)
        cwg = np.array(canon_w[cs, 0, :], np.float32, copy=True)
        cwg[:, KCONV - 1] += 1.0      # fold residual into last conv tap
        per_g.append({
            "WqT": _bf16(Wq[cs].T),
            "WkT": _bf16(Wk[cs].T),
            "WvTa": _bf16(WvTa),
            "WgT": _bf16(Wg[hs].T),
            "gbn": np.ascontiguousarray(-gate_b[hs]).reshape(HPG, 1),
            "WoT": _bf16(Wo[:, cs].T),
            "cw": cwg,
            "cb": np.ascontiguousarray(canon_bias[cs]).reshape(CPG, 1),
        })
    return per_b, per_g


LAST_EXEC_NS = None
LAST_RESULTS = None


def kernel(**inputs):
    global LAST_EXEC_NS, LAST_RESULTS
    from concourse.bass_utils import run_bass_kernel_spmd

    nc = _get_program()
    per_b, per_g = host_prep(inputs)
    in_maps = []
    for core in range(8):
        b, g = divmod(core, G)
        m = {}
        m.update(per_b[b])
        m.update(per_g[g])
        in_maps.append(m)

    trace = bool(int(os.environ.get("KERNEL_TRACE", "0")))
    kw = {}
    if trace:
        tcores = os.environ.get("KERNEL_TRACE_CORES", "0")
        kw = dict(
            trace=True,
            trace_cores=[int(c) for c in tcores.split(",")],
            tmpdir=os.environ.get("KERNEL_TRACE_DIR", None),
        )
    res = run_bass_kernel_spmd(nc, in_maps, core_ids=list(range(8)), **kw)
    LAST_EXEC_NS = res.exec_time_ns
    LAST_RESULTS = res
    outp = np.zeros((B, T, C), np.float32)
    for core in range(8):
        b = core // G
        outp[b] += np.asarray(res.results[core]["out"], np.float32)
    return outp
